# revision 33
# baseline (speedup 1.0000x reference)
"""ButterflyBlock sparse-attention kernel for 8 Trainium2 NeuronCores.

Full inputs in, full output out. The P*B = 32 butterfly blocks are
data-parallel: 4 blocks per core, QKVO weights replicated (streamed from
HBM per block), chunk gather/scatter done host-side in numpy.

Hardcoded problem shape: x [4, 4096, 1024], D=1024, H=16 heads, dh=64,
CHUNK=256 -> C=16 chunks, pairs a < a^(1<<layer_bit), blocks of L=512.
"""

import sys

sys.path.insert(0, "/root/.axon_site/_ro/trn_rl_repo")
sys.path.insert(0, "/opt/trn_rl_repo")

import ml_dtypes
import numpy as np

import concourse.bass as bass
import concourse.bacc as bacc
import concourse.mybir as mybir
import concourse.tile as tile
from concourse.bass_utils import run_bass_kernel_spmd

F32 = mybir.dt.float32
F32R = mybir.dt.float32r
BF16 = mybir.dt.bfloat16

B, N, D = 4, 4096, 1024
H, DH = 16, 64
CHUNK = 256
L = 2 * CHUNK          # 512 tokens per block
NBLK = 4               # blocks per core
NCORES = 8
KC = D // 128          # 8 contraction chunks
LC = L // 128          # 4 token chunks
EXP_FUNC = mybir.ActivationFunctionType.Exp
DEBUG = False

# v_sb free layout per m-chunk: 16 head-blocks of 128 cols each;
# even head: [v_h(64)|ones(64)], odd head: [ones(64)|v_h(64)]
VW = H * 128           # 2048


def _build_nc(has_bq, has_bk, has_bv):
    nc = bacc.Bacc("TRN2", target_bir_lowering=False, debug=False)

    zt = nc.dram_tensor("zt", [NBLK, D, L], BF16, kind="ExternalInput")
    wq = nc.dram_tensor("wq", [D, D], BF16, kind="ExternalInput")
    wk = nc.dram_tensor("wk", [D, D], BF16, kind="ExternalInput")
    wv = nc.dram_tensor("wv", [D, D], BF16, kind="ExternalInput")
    wo = nc.dram_tensor("wo", [D, D], BF16, kind="ExternalInput")
    ones = nc.dram_tensor("ones", [128, 64], BF16, kind="ExternalInput")
    y = nc.dram_tensor("y", [NBLK, L, D], F32, kind="ExternalOutput")
    dbg = {}
    if DEBUG:
        dbg["q"] = nc.dram_tensor("dbg_q", [128, KC, L], BF16, kind="ExternalOutput")
        dbg["k"] = nc.dram_tensor("dbg_k", [128, KC, L], BF16, kind="ExternalOutput")
        dbg["v"] = nc.dram_tensor("dbg_v", [128, LC, VW], BF16, kind="ExternalOutput")
        dbg["p0"] = nc.dram_tensor("dbg_p0", [128, LC, 512], BF16, kind="ExternalOutput")
        dbg["p1"] = nc.dram_tensor("dbg_p1", [128, LC, 512], BF16, kind="ExternalOutput")
        dbg["u"] = nc.dram_tensor("dbg_u", [128, KC, L], BF16, kind="ExternalOutput")
        dbg["r0"] = nc.dram_tensor("dbg_r0", [128, 512], F32, kind="ExternalOutput")
        dbg["ua"] = nc.dram_tensor("dbg_ua", [128, 512], F32, kind="ExternalOutput")
        dbg["ub"] = nc.dram_tensor("dbg_ub", [128, 512], F32, kind="ExternalOutput")
    bq = bk = bv = None
    if has_bq:
        bq = nc.dram_tensor("bq", [128, KC], F32, kind="ExternalInput")
    if has_bk:
        bk = nc.dram_tensor("bk", [128, KC], F32, kind="ExternalInput")
    if has_bv:
        bv = nc.dram_tensor("bv", [128, KC], F32, kind="ExternalInput")

    with tile.TileContext(nc) as tc:
        with (
            tc.tile_pool(name="wpool", bufs=3) as wpool,
            tc.tile_pool(name="zpool", bufs=3) as zpool,
            tc.tile_pool(name="qkpool", bufs=4) as qkpool,
            tc.tile_pool(name="vpool", bufs=2) as vpool,
            tc.tile_pool(name="ppool", bufs=3) as ppool,
            tc.tile_pool(name="upool", bufs=2) as upool,
            tc.tile_pool(name="rpool", bufs=3) as rpool,
            tc.tile_pool(name="ysb", bufs=3) as ypool,
            tc.tile_pool(name="bias", bufs=1) as bpool,
            tc.tile_pool(name="mmps", bufs=4, space="PSUM") as mmps,
            tc.tile_pool(name="scps", bufs=1, space="PSUM") as scps,
                    ):
            bq_sb = bk_sb = bv_sb = None
            if has_bq:
                bq_sb = bpool.tile([128, KC], F32)
                nc.sync.dma_start(bq_sb[:], bq[:])
            if has_bk:
                bk_sb = bpool.tile([128, KC], F32)
                nc.sync.dma_start(bk_sb[:], bk[:])
            if has_bv:
                bv_sb = bpool.tile([128, KC], F32)
                nc.sync.dma_start(bv_sb[:], bv[:])

            pending_wo = None

            def emit_wo(pending):
                pblk, pu_sb, pwo_sb = pending
                for lc in range(LC):
                    for eh in range(2):
                        ps = mmps.tile([128, 512], F32, tag="ps", name="ps")
                        for dc in range(KC):
                            nc.tensor.matmul(
                                ps[:],
                                pu_sb[:, dc, lc * 128:(lc + 1) * 128].opt(),
                                pwo_sb[:, dc, eh * 512:(eh + 1) * 512].opt(),
                                start=(dc == 0),
                                stop=(dc == KC - 1),
                            )
                        y_sb = ypool.tile([128, 512], F32, tag="y_sb", name="y_sb")
                        nc.vector.tensor_copy(y_sb[:], ps[:])
                        nc.sync.dma_start(
                            y[pblk, lc * 128:(lc + 1) * 128,
                              eh * 512:(eh + 1) * 512],
                            y_sb[:],
                        )

            for blk in range(NBLK):
                # ---- load z^T  [128p, kc, l]
                zt_sb = zpool.tile([128, KC, L], BF16)
                zt_r = zt[blk].rearrange("(kc p) l -> p kc l", p=128)
                for kc in range(KC):
                    if blk == 0 and kc < 2:
                        # first-needed chunks: split across partition ranges so
                        # the descriptors spread over multiple DMA queues
                        for pr in range(4):
                            nc.sync.dma_start(
                                zt_sb[pr * 32:(pr + 1) * 32, kc, :],
                                zt_r[pr * 32:(pr + 1) * 32, kc, :])
                    else:
                        nc.sync.dma_start(zt_sb[:, kc, :], zt_r[:, kc, :])

                # ---- Q^T / K^T projections -> bf16 [128p, kc(dout), l]
                qk_tiles = []
                for name, w_dram, b_sb in (("q", wq, bq_sb), ("k", wk, bk_sb)):
                    w_sb = wpool.tile([128, KC, D], BF16, tag="w")
                    w_r = w_dram.rearrange("(kc p) d -> p kc d", p=128)
                    for kc in range(KC):
                        if blk == 0 and name == "q" and kc < 2:
                            for pr in range(4):
                                nc.sync.dma_start(
                                    w_sb[pr * 32:(pr + 1) * 32, kc, :],
                                    w_r[pr * 32:(pr + 1) * 32, kc, :])
                        else:
                            nc.sync.dma_start(w_sb[:, kc, :], w_r[:, kc, :])
                    out_sb = qkpool.tile([128, KC, L], BF16, tag="qk")
                    for dc in range(KC):
                        ps = mmps.tile([128, L], F32)
                        for kc in range(KC):
                            nc.tensor.matmul(
                                ps[:],
                                w_sb[:, kc, dc * 128:(dc + 1) * 128].opt(),
                                zt_sb[:, kc, :].opt(),
                                start=(kc == 0),
                                stop=(kc == KC - 1),
                            )
                        if b_sb is not None:
                            nc.scalar.activation(
                                out_sb[:, dc, :], ps[:],
                                mybir.ActivationFunctionType.Identity,
                                bias=b_sb[:, dc:dc + 1], scale=1.0,
                            )
                        else:
                            nc.vector.tensor_copy(out_sb[:, dc, :], ps[:])
                    qk_tiles.append(out_sb)
                    if pending_wo is not None:
                        emit_wo(pending_wo)
                        pending_wo = None
                q_sb, k_sb = qk_tiles
                if DEBUG and blk == 0:
                    nc.sync.dma_start(dbg["q"][:], q_sb[:])
                    nc.sync.dma_start(dbg["k"][:], k_sb[:])

                # ---- V projection -> natural layout f32r with ones margins
                wv_sb = wpool.tile([128, KC, D], BF16, tag="w")
                wv_r = wv.rearrange("(kc p) d -> p kc d", p=128)
                for kc in range(KC):
                    nc.sync.dma_start(wv_sb[:, kc, :], wv_r[:, kc, :])
                # per m-chunk layout: 16 head blocks of 128 cols;
                # even head h: [v_h(64) | ones(64)], odd head h: [ones(64) | v_h(64)]
                v_sb = vpool.tile([128, LC, VW], BF16)
                ones_b = bass.AP(
                    tensor=ones[:].tensor, offset=ones[:].offset,
                    ap=[list(ones[:].ap[0]), [0, H // 2], [1, 64]],
                )
                for lc in range(LC):
                    base = v_sb[:, lc, :]
                    for par, ooff in ((0, 64), (1, 128)):
                        dst = bass.AP(
                            tensor=base.tensor, offset=base.offset + ooff,
                            ap=[list(base.ap[0]), [256, H // 2], [1, 64]],
                        )
                        nc.sync.dma_start(dst, ones_b)
                for lc in range(LC):
                    for nh in range(2):
                        ps = mmps.tile([128, 512], F32)
                        for kc in range(KC):
                            nc.tensor.matmul(
                                ps[:],
                                zt_sb[:, kc, lc * 128:(lc + 1) * 128].opt(),
                                wv_sb[:, kc, nh * 512:(nh + 1) * 512].opt(),
                                start=(kc == 0),
                                stop=(kc == KC - 1),
                            )
                        base = v_sb[:, lc, :]
                        for par in range(2):  # even / odd heads of this half
                            dst = bass.AP(
                                tensor=base.tensor,
                                offset=base.offset + (nh * 8 + par) * 128 + par * 64,
                                ap=[list(base.ap[0]), [256, 4], [1, 64]],
                            )
                            src = bass.AP(
                                tensor=ps.tensor,
                                offset=ps[:].offset + par * 64,
                                ap=[list(ps[:].ap[0]), [128, 4], [1, 64]],
                            )
                            nc.vector.tensor_copy(dst, src)

                if DEBUG and blk == 0:
                    nc.sync.dma_start(dbg["v"][:], v_sb[:])
                # ---- attention, head by head
                wo_sb = wpool.tile([128, KC, D], BF16, tag="w")
                wo_r = wo.rearrange("(kc p) d -> p kc d", p=128)
                for kc in range(KC):
                    nc.sync.dma_start(wo_sb[:, kc, :], wo_r[:, kc, :])
                u_sb = upool.tile([128, KC, L], BF16)
                for c in range(H // 2):
                    # scores^T for the head pair, interleaved across PE
                    # row-halves so consecutive LDWEIGHTS overlap the
                    # in-flight matmul on the other half; 2-bank tiles per
                    # half-pair keep PSUM within budget.
                    p_even = ppool.tile([128, LC, 512], BF16, tag="pe", name="p_even")
                    p_odd = ppool.tile([128, LC, 512], BF16, tag="po", name="p_odd")
                    p_pair = [p_even, p_odd]
                    for mg in range(2):          # m-chunk group {0,1},{2,3}
                        sc_e = scps.tile([128, 2, 512], F32, tag="sce")
                        sc_o = scps.tile([128, 2, 512], F32, tag="sco")
                        for i in range(2):
                            mc = 2 * mg + i
                            for par, sc in ((0, sc_e), (1, sc_o)):
                                half = par * 64
                                nc.tensor.matmul(
                                    sc[:, i, :],
                                    k_sb[half:half + 64, c,
                                         mc * 128:(mc + 1) * 128].opt(),
                                    q_sb[half:half + 64, c, :].opt(),
                                    start=True, stop=True,
                                )
                        nc.scalar.activation(
                            p_pair[0][:, 2 * mg:2 * mg + 2, :], sc_e[:], EXP_FUNC)
                        nc.scalar.activation(
                            p_pair[1][:, 2 * mg:2 * mg + 2, :], sc_o[:], EXP_FUNC)
                    u_ps_pair = []
                    for par in range(2):  # even / odd head of the pair
                        h = 2 * c + par
                        p_sb = p_pair[par]
                        if DEBUG and blk == 0 and h in (0, 1):
                            nc.sync.dma_start(dbg["p%d" % h][:], p_sb[:])
                        # PV with ones-augmented stationary operand:
                        #  even head block [v|ones] -> u rows 0:64, S rows 64:128
                        #  odd head block [ones|v]  -> S rows 0:64, u rows 64:128
                        u_ps = mmps.tile([128, 512], F32, tag="ps", name="ps")
                        for mc in range(LC):
                            nc.tensor.matmul(
                                u_ps[:],
                                v_sb[:, mc, h * 128:(h + 1) * 128].opt(),
                                p_sb[:, mc, :].opt(),
                                start=(mc == 0), stop=(mc == LC - 1),
                            )
                        u_ps_pair.append(u_ps)
                    ups_a, ups_b = u_ps_pair
                    if DEBUG and blk == 0 and c == 0:
                        tmpa = rpool.tile([128, 512], F32, tag="dbgtmp")
                        nc.vector.tensor_copy(tmpa[:], ups_a[:])
                        nc.sync.dma_start(dbg["ua"][:], tmpa[:])
                        tmpb = rpool.tile([128, 512], F32, tag="dbgtmp")
                        nc.vector.tensor_copy(tmpb[:], ups_b[:])
                        nc.sync.dma_start(dbg["ub"][:], tmpb[:])
                    # custom DVE ops (recip) only work at partition base 0,
                    # so S_A is first cross-copied down to base 0; the
                    # standard TT mul handles the in1 base crossing for B.
                    tmp = rpool.tile([64, 512], F32, tag="rtmp")
                    nc.vector.tensor_copy(tmp[0:64, :], ups_a[64:128, :])
                    r_a = rpool.tile([64, 512], F32, tag="ra")
                    nc.vector.reciprocal_approx_fast(r_a[0:64, :], tmp[0:64, :])
                    r_b = rpool.tile([64, 512], F32, tag="rb")
                    nc.vector.reciprocal_approx_fast(r_b[0:64, :], ups_b[0:64, :])
                    nc.vector.tensor_mul(
                        u_sb[0:64, c, :], ups_a[0:64, :], r_a[0:64, :]
                    )
                    nc.vector.tensor_mul(
                        u_sb[64:128, c, :], ups_b[64:128, :], r_b[0:64, :]
                    )
                    if DEBUG and blk == 0 and c == 0:
                        nc.sync.dma_start(dbg["r0"][0:64, :], r_a[0:64, :])
                        nc.sync.dma_start(dbg["r0"][64:128, :], r_b[0:64, :])
                    if has_bv:
                        nc.vector.tensor_scalar_add(
                            u_sb[:, c, :], u_sb[:, c, :], bv_sb[:, c:c + 1]
                        )

                if DEBUG and blk == 0:
                    nc.sync.dma_start(dbg["u"][:], u_sb[:])
                # Wo for this block is emitted at the start of the next
                # block (emit_wo) so the PE fills the softmax-chain wait
                # with next-block projection work.
                pending_wo = (blk, u_sb, wo_sb)

            emit_wo(pending_wo)
    nc.finalize()
    return nc


_NC_CACHE = {}


def _get_nc(flags):
    if flags not in _NC_CACHE:
        _NC_CACHE[flags] = _build_nc(*flags)
    return _NC_CACHE[flags]


def _prep(x, Wq, bq, Wk, bk, Wv, bv, Wo, bo, layer_bit):
    x = np.asarray(x, dtype=np.float32)
    C = N // CHUNK
    ids = np.arange(C)
    partner = ids ^ (1 << int(layer_bit))
    a_idx = ids[ids < partner]
    b_idx = partner[ids < partner]
    P = a_idx.shape[0]

    xr = x.reshape(B, C, CHUNK, D)
    blocks = np.concatenate([xr[:, a_idx], xr[:, b_idx]], axis=2)  # [B,P,L,D]
    blocks = np.ascontiguousarray(
        blocks.transpose(1, 0, 3, 2).reshape(P * B, D, L).astype(ml_dtypes.bfloat16)
    )  # z^T per block
    scale = np.float32(1.0 / np.sqrt(DH))

    def chunkify(vec):  # [D] -> [128, KC] chunk-major per-partition scalars
        return np.ascontiguousarray(
            np.asarray(vec, np.float32).reshape(KC, 128).T
        )

    bf = ml_dtypes.bfloat16
    base = {
        "wq": np.ascontiguousarray((np.asarray(Wq, np.float32) * scale).astype(bf)),
        "wk": np.ascontiguousarray(np.asarray(Wk, np.float32).astype(bf)),
        "wv": np.ascontiguousarray(np.asarray(Wv, np.float32).astype(bf)),
        "wo": np.ascontiguousarray(np.asarray(Wo, np.float32).astype(bf)),
        "ones": np.ones((128, 64), bf),
    }
    has_bq = bool(np.any(np.asarray(bq))) if bq is not None else False
    has_bk = bool(np.any(np.asarray(bk))) if bk is not None else False
    has_bv = bool(np.any(np.asarray(bv))) if bv is not None else False
    if has_bq:
        base["bq"] = chunkify(np.asarray(bq, np.float32) * scale)
    if has_bk:
        base["bk"] = chunkify(bk)
    if has_bv:
        base["bv"] = chunkify(bv)

    in_maps = []
    for core in range(NCORES):
        m = dict(base)
        m["zt"] = blocks[core * NBLK:(core + 1) * NBLK]
        in_maps.append(m)
    return in_maps, (has_bq, has_bk, has_bv), (a_idx, b_idx, P)


def _gather(results, idxs, bo):
    a_idx, b_idx, P = idxs
    yb = np.concatenate([r["y"] for r in results], axis=0)  # [P*B, L, D]
    yb = yb.reshape(P, B, 2, CHUNK, D)
    out = np.empty((B, N // CHUNK, CHUNK, D), np.float32)
    out[:, a_idx] = yb[:, :, 0].transpose(1, 0, 2, 3)
    out[:, b_idx] = yb[:, :, 1].transpose(1, 0, 2, 3)
    out = out.reshape(B, N, D)
    bo = np.asarray(bo, np.float32) if bo is not None else None
    if bo is not None and np.any(bo):
        out = out + bo
    return out


def _run(inputs, trace=False):
    in_maps, flags, idxs = _prep(
        inputs["x"], inputs["Wq"], inputs.get("bq"), inputs["Wk"],
        inputs.get("bk"), inputs["Wv"], inputs.get("bv"), inputs["Wo"],
        inputs.get("bo"), inputs["layer_bit"],
    )
    nc = _get_nc(flags)
    res = run_bass_kernel_spmd(nc, in_maps, list(range(NCORES)), trace=trace)
    out = _gather(res.results, idxs, inputs.get("bo"))
    return out, res


def kernel(**inputs):
    out, _ = _run(inputs, trace=False)
    return out


def kernel_traced(**inputs):
    out, res = _run(inputs, trace=True)
    return out, res
